# revision 2
# baseline (speedup 1.0000x reference)
"""CRF head kernel for Trainium2 (Bass/Tile), 8-core data-parallel.

Computes: out[b, t, :] = x[b, t, :] + transitions[argmax(x[b, t, :]), :]
for x of shape [128, 1024, 256] f32 and transitions [256, 256] f32.

Sharding: batch dim split across 8 NeuronCores (16 batches / core).
Per core: 16*1024 = 16384 rows, processed in megatiles of 1024 rows laid
out as [128 partitions, 8 groups, 256 tags] (each partition holds 8
consecutive rows -> contiguous 8KB DMA descriptors per partition).

Per megatile:
  1. HWDGE load 1MB from HBM.
  2. VectorE max (top-8) + max_index per group -> argmax index per row
     (uint16).
  3. Index re-layout for the gather ucode: strided store of the argmax
     lane to a DRAM scratch (p-major), then 8 small loads that build the
     wrapped [16, n/16] int16 index tile replicated across the 8 GpSimd
     core partition-groups.
  4. One SWDGE dma_gather pulls transitions[idx] rows (1KB each) from HBM
     into an SBUF tile.
  5. VectorE add, HWDGE store 1MB back to HBM.
"""

import sys

for _p in ("/opt/trn_rl_repo",):
    if _p not in sys.path:
        sys.path.append(_p)

import numpy as np

import concourse.bass as bass  # noqa: F401  (AP helpers)
import concourse.bacc as bacc
import concourse.mybir as mybir
import concourse.tile as tile
import concourse.bass_utils as bass_utils

N_CORES = 8
B, T, TAGS = 128, 1024, 256
R = (B // N_CORES) * T          # rows per core = 16384
P = 128                         # SBUF partitions
G = 8                           # rows per partition per megatile
ROWS_PER_MT = P * G             # 1024
M = R // ROWS_PER_MT            # 16 megatiles per core
S = ROWS_PER_MT // 16           # wrapped idx columns = 64

_CACHE = {}


def _build():
    nc = bacc.Bacc("TRN2", target_bir_lowering=False, debug=False)

    x = nc.dram_tensor("x", [R, TAGS], mybir.dt.float32, kind="ExternalInput")
    t = nc.dram_tensor("t", [TAGS, TAGS], mybir.dt.float32, kind="ExternalInput")
    y = nc.dram_tensor("y", [R, TAGS], mybir.dt.float32, kind="ExternalOutput")

    # megatile m, partition p holds rows m*1024 + p*G .. +G-1 (contiguous)
    xv = x.ap().rearrange("(m p g) d -> m p (g d)", p=P, g=G)
    yv = y.ap().rearrange("(m p g) d -> m p (g d)", p=P, g=G)

    with tile.TileContext(nc) as tc:
        with (
            tc.tile_pool(name="xp", bufs=3) as xp,
            tc.tile_pool(name="gp", bufs=3) as gp,
            tc.tile_pool(name="sp", bufs=3) as sp,
            tc.tile_pool(name="dp", bufs=3, space="DRAM") as dp,
        ):
            for m in range(M):
                x_t = xp.tile([P, G * TAGS], mybir.dt.float32, tag="x",
                              name=f"x_{m}")
                nc.sync.dma_start(out=x_t[:], in_=xv[m])

                mx8 = sp.tile([P, G * 8], mybir.dt.float32, tag="mx",
                              name=f"mx_{m}")
                idx8 = sp.tile([P, G * 8], mybir.dt.uint16, tag="idx",
                               name=f"idx_{m}")
                for g in range(G):
                    seg = slice(g * TAGS, (g + 1) * TAGS)
                    e8 = slice(g * 8, (g + 1) * 8)
                    nc.vector.max(out=mx8[:, e8], in_=x_t[:, seg])
                    nc.vector.max_index(out=idx8[:, e8], in_max=mx8[:, e8],
                                        in_values=x_t[:, seg])

                # scratch[p*G + c] = idx8[p, c*8] (argmax of row p*G+c)
                scr = dp.tile([ROWS_PER_MT], mybir.dt.uint16, tag="scr",
                              name=f"scr_{m}")
                nc.scalar.dma_start(
                    out=scr[:].rearrange("(p c) -> p c", p=P),
                    in_=idx8[:].rearrange("p (c e) -> p c e", e=8)[:, :, 0:1],
                )
                # wrapped[16k+q, c*8+r] = scratch[(r*16+q)*G + c], k=0..7
                wrapped = sp.tile([P, S], mybir.dt.uint16, tag="wr",
                                  name=f"wr_{m}")
                scr_v = scr[:].rearrange("(r q c) -> q c r", q=16, c=G)
                wv = wrapped[:].rearrange("(k q) (c r) -> k q c r", k=8, r=8)
                for k in range(8):
                    nc.scalar.dma_start(out=wv[k], in_=scr_v)

                g_t = gp.tile([P, G * TAGS], mybir.dt.float32, tag="g",
                              name=f"g_{m}")
                nc.gpsimd.dma_gather(
                    out_ap=g_t[:].rearrange("p (c d) -> p c d", d=TAGS),
                    in_ap=t.ap(),
                    idxs_ap=wrapped[:].bitcast(mybir.dt.int16),
                    num_idxs=ROWS_PER_MT,
                    num_idxs_reg=ROWS_PER_MT,
                    elem_size=TAGS,
                )
                nc.vector.tensor_add(out=x_t[:], in0=x_t[:], in1=g_t[:])
                nc.sync.dma_start(out=yv[m], in_=x_t[:])

    nc.compile()
    return nc


def get_nc():
    if "nc" not in _CACHE:
        _CACHE["nc"] = _build()
    return _CACHE["nc"]


def kernel(launch_matrix, transitions):
    launch = np.ascontiguousarray(np.asarray(launch_matrix, dtype=np.float32))
    trans = np.ascontiguousarray(np.asarray(transitions, dtype=np.float32))
    assert launch.shape == (B, T, TAGS), launch.shape
    assert trans.shape == (TAGS, TAGS), trans.shape

    nc = get_nc()
    shards = launch.reshape(N_CORES, R, TAGS)
    in_maps = [{"x": shards[c], "t": trans} for c in range(N_CORES)]
    res = bass_utils.run_bass_kernel_spmd(nc, in_maps,
                                          core_ids=list(range(N_CORES)))
    _CACHE["last_results"] = res
    out = np.concatenate([res.results[c]["y"] for c in range(N_CORES)], axis=0)
    return out.reshape(B, T, TAGS)
